# revision 35
# baseline (speedup 1.0000x reference)
"""R-GCN (2-layer basis-decomposition GCN) on 8 Trainium2 NeuronCores.

The end-to-end time here is dominated by host->device input transfer over
the (axon-tunneled) PJRT link, so the design minimizes transferred bytes
and lets the device do all the math.

Strategy (1D node partition, per sharding hint):
- Nodes sharded 1024/core. The feature shard is sent TRANSPOSED as int12
  fixed point, split into an int8 hi plane + packed 4-bit lo plane
  (12MB/core vs 32MB fp32; the Gaussian hi plane additionally wire-
  compresses to ~0.85). The device reassembles q = 16*hi + nibble into
  fp16 (exact: |q| <= 2048) and runs the support matmul in fp16 with
  f32 PSUM accumulate. The dequant scale delta is folded into the
  layer-1 edge weights (aggregation is linear in w; tanh comes after),
  so no extra device pass is needed; layer 2 uses unscaled weights.
- The small basis combinations V1 = Wc1 x W1 ([8192, 256]) and
  V2 = Wc2 x W2 ([64, 128]) are computed on host. V1 is row-sharded
  (fp16, 0.5MB/core) and AllGathered on device; V2 is replicated (32KB).
- sup1 tables are AllGathered to Shared DRAM ([8192, 256] f32).
- Edges sharded by destination node, bucketed per (dst-block of 128,
  relation), padded to 128-edge chunks (pad: src=0, w=0). Edge index
  stream is sent untiled ([16, tot/16] int16) and replicated to the 128
  partitions on device (dma_gather wants indices wrapped in 16 partitions
  replicated across the 8 gpsimd cores); local dst rows as uint8;
  weights as fp16.
- Messages gathered with gpsimd.dma_gather (256B rows) landing as
  [128 edges (partitions), 64 feats] — directly the matmul moving operand.
- segment_sum = one-hot matmul: stationary [128e,128d] weighted one-hot
  built by one DVE tensor_scalar (iota is_equal dst) * w; PSUM
  accumulates per block.
- Layer 2 identical with a [8192,192] padded table; classifier on PE.

Accuracy: int12 features + fp16 V1 give rel err 5.5e-3 vs the f32
reference (gate 2e-2); the device pipeline reproduces the numpy
emulation of this quantization to 4 digits.
"""
import sys
import numpy as np

sys.path.insert(0, "/opt/trn_rl_repo")
import jax  # noqa: E402

# cache the XLA wrapper executable: run_bass_kernel_spmd re-jits per call
jax.config.update("jax_compilation_cache_dir", "/tmp/jaxcache")
jax.config.update("jax_persistent_cache_min_entry_size_bytes", 0)
jax.config.update("jax_persistent_cache_min_compile_time_secs", 0.0)

from concourse import bacc, bass, mybir, tile  # noqa: E402
from concourse.bass_utils import run_bass_kernel_spmd  # noqa: E402

FP16 = mybir.dt.float16
F32 = mybir.dt.float32
I16 = mybir.dt.int16
I32 = mybir.dt.int32
I8 = mybir.dt.int8
U8 = mybir.dt.uint8
NPFP16 = np.float16

N = 8192
S = 4
E = 262144
H = 64
F = 32
C = 2
NCORES = 8
NPC = N // NCORES      # 1024 nodes per core
NB = NPC // 128        # 8 dst blocks per core
KCH = N // 128         # 64 contraction chunks for layer 1
T2COLS = 192           # layer-2 table padded cols (768B rows)

# The axon tunnel parallelizes transfers PER JIT ARGUMENT (~41MB/s each,
# ~linear aggregate scaling to 64+ streams) — so large inputs are split
# into many small tensors to ride parallel streams.
NFH = 32               # fhi row-split count (2MB global chunks)
NFL = 16               # flo row-split count
RH = N // NFH          # 256 rows per fhi chunk (= 2 k-chunks)
RL = N // NFL          # 512 rows per flo chunk (= 4 k-chunks)


def build_program(cnt, delta):
    """cnt: [NB][S] padded edge counts (identical across cores).
    delta: int12 feature quantization step (folded into layer-1 edge
    weights; layer 2 uses the unscaled weights)."""
    nc = bacc.Bacc(None)

    tot = sum(cnt[b][s] for b in range(NB) for s in range(S))
    ncol = tot // 128

    fhi = [
        nc.dram_tensor(f"fhi{j:02d}", [RH, NPC], I8, kind="ExternalInput")
        for j in range(NFH)
    ]
    flo = [
        nc.dram_tensor(f"flo{j:02d}", [RL, NPC // 2], U8, kind="ExternalInput")
        for j in range(NFL)
    ]
    v1s = [
        nc.dram_tensor(f"v1s{j}", [NPC // 2, 4 * H], FP16, kind="ExternalInput")
        for j in range(2)
    ]
    v2c = nc.dram_tensor("v2c", [H, 4 * F], F32, kind="ExternalInput")
    wclf = nc.dram_tensor("wclf", [F, C], F32, kind="ExternalInput")
    bc = nc.dram_tensor("bc", [C, 1], F32, kind="ExternalInput")
    ecols = tot // 16
    eh = ecols // 2
    eidx = [
        nc.dram_tensor("eidx0", [16, eh], I16, kind="ExternalInput"),
        nc.dram_tensor("eidx1", [16, ecols - eh], I16, kind="ExternalInput"),
    ]
    edst8 = nc.dram_tensor("edst8", [128, ncol], U8, kind="ExternalInput")
    nch_half = ncol // 2
    ew16 = [
        nc.dram_tensor("ew16_0", [128, nch_half], FP16, kind="ExternalInput"),
        nc.dram_tensor("ew16_1", [128, ncol - nch_half], FP16, kind="ExternalInput"),
    ]
    out = nc.dram_tensor("out", [C, NPC], F32, kind="ExternalOutput")

    agv1 = nc.dram_tensor("agv1", [NPC, 4 * H], FP16)
    tbv1 = nc.dram_tensor("tbv1", [N, 4 * H], FP16, addr_space="Shared")
    ag1_in = nc.dram_tensor("ag1_in", [NPC, 4 * H], F32)
    table1 = nc.dram_tensor("table1", [N, 4 * H], F32, addr_space="Shared")
    ag2_in = nc.dram_tensor("ag2_in", [NPC, T2COLS], F32)
    table2 = nc.dram_tensor("table2", [N, T2COLS], F32, addr_space="Shared")

    rg = [list(range(NCORES))]

    with tile.TileContext(nc) as tc:
        with tc.tile_pool(name="const", bufs=1) as cp:
            # ---- constants ----
            iota_i = cp.tile([128, 128], I32)
            nc.gpsimd.iota(iota_i, pattern=[[1, 128]], base=0, channel_multiplier=0)
            iota_f = cp.tile([128, 128], F32)
            nc.vector.tensor_copy(iota_f, iota_i)
            idn_i = cp.tile([128, 128], I32)
            nc.gpsimd.iota(idn_i, pattern=[[1, 128]], base=0, channel_multiplier=-1)
            ident = cp.tile([128, 128], F32)
            nc.vector.tensor_scalar(
                ident, idn_i, 0, None, mybir.AluOpType.is_equal
            )

            # edge streams: replicate idx block to all 8 gpsimd stripes
            eidx_sb = cp.tile([128, tot // 16], I16)
            for i in range(8):
                nc.sync.dma_start(eidx_sb[16 * i : 16 * (i + 1), :eh], eidx[0][:, :])
                nc.sync.dma_start(eidx_sb[16 * i : 16 * (i + 1), eh:], eidx[1][:, :])
            edst8_sb = cp.tile([128, ncol], U8)
            nc.sync.dma_start(edst8_sb, edst8[:, :])
            edst_sb = cp.tile([128, ncol], F32)
            nc.vector.tensor_copy(edst_sb, edst8_sb)
            ew16_sb = cp.tile([128, ncol], FP16)
            nc.sync.dma_start(ew16_sb[:, :nch_half], ew16[0][:, :])
            nc.sync.dma_start(ew16_sb[:, nch_half:], ew16[1][:, :])
            # layer-1 weights absorb the int12 dequant scale; layer 2 unscaled
            ew1_sb = cp.tile([128, ncol], F32)
            nc.vector.tensor_scalar(
                ew1_sb, ew16_sb, float(delta), None, mybir.AluOpType.mult
            )
            ew2_sb = cp.tile([128, ncol], F32)
            nc.vector.tensor_copy(ew2_sb, ew16_sb)

            x1_sb = cp.tile([128, NB, H], F32)
            x1t_sb = cp.tile([H, NPC], F32)
            x2_sb = cp.tile([128, NB, F], F32)
            v2_sb = cp.tile([H, 4 * F], F32)
            nc.sync.dma_start(v2_sb, v2c[:, :])
            wclf_sb = cp.tile([F, C], F32)
            nc.sync.dma_start(wclf_sb, wclf[:, :])
            bclf_sb = cp.tile([C, 1], F32)
            nc.sync.dma_start(bclf_sb, bc[:, :])
            out_sb = cp.tile([C, NPC], F32)

            # ---- phase 0: AllGather V1 (host-combined, row-sharded) ----
            v1b = cp.tile([128, NB, 4 * H], FP16)
            for b in range(NB):
                src = v1s[b // (NB // 2)]
                r = (b % (NB // 2)) * 128
                nc.sync.dma_start(v1b[:, b, :], src[r : r + 128, :])
            for b in range(NB):
                nc.sync.dma_start(agv1[128 * b : 128 * (b + 1), :], v1b[:, b, :])
            nc.gpsimd.collective_compute(
                "AllGather", mybir.AluOpType.bypass, replica_groups=rg,
                ins=[agv1[:]], outs=[tbv1[:]],
            )
            v1 = cp.tile([128, KCH, 4 * H], FP16)
            for k in range(KCH):
                nc.sync.dma_start(v1[:, k, :], tbv1[128 * k : 128 * (k + 1), :])

            # ---- phase 1: support matmul sup1 = featT.T @ V1cat ----
            with (
                tc.tile_pool(name="ftp", bufs=3) as ftp,
                tc.tile_pool(name="spp", bufs=1, space="PSUM") as spp,
                tc.tile_pool(name="ssb", bufs=2) as ssb,
            ):
                ps = [
                    spp.tile([128, 4 * H], F32, tag=f"ps{b}", name=f"ps{b}")
                    for b in range(NB)
                ]
                hw = NPC // 2
                for k in range(KCH):
                    hik = ftp.tile([128, NPC], I8, tag="hik")
                    rh = (k * 128) % RH
                    nc.sync.dma_start(hik, fhi[(k * 128) // RH][rh : rh + 128, :])
                    lok = ftp.tile([128, hw], U8, tag="lok")
                    rl = (k * 128) % RL
                    nc.sync.dma_start(lok, flo[(k * 128) // RL][rl : rl + 128, :])
                    # reassemble q = 16*hi + nibble into fp16 (q exact: |q|<=2048)
                    nib = ftp.tile([128, NPC], U8, tag="nib")
                    nc.vector.tensor_scalar(
                        nib[:, :hw], lok, 15, None, mybir.AluOpType.bitwise_and
                    )
                    nc.vector.tensor_scalar(
                        nib[:, hw:], lok, 4, None,
                        mybir.AluOpType.logical_shift_right,
                    )
                    ftk = ftp.tile([128, NPC], FP16, tag="ftk")
                    nc.vector.scalar_tensor_tensor(
                        ftk, hik, 16.0, nib,
                        mybir.AluOpType.mult, mybir.AluOpType.add,
                    )
                    for b in range(NB):
                        nc.tensor.matmul(
                            ps[b], lhsT=ftk[:, 128 * b : 128 * (b + 1)],
                            rhs=v1[:, k, :],
                            start=(k == 0), stop=(k == KCH - 1),
                        )
                for b in range(NB):
                    s_sb = ssb.tile([128, 4 * H], F32, tag="ssb")
                    nc.any.tensor_copy(s_sb, ps[b])
                    nc.sync.dma_start(ag1_in[128 * b : 128 * (b + 1), :], s_sb)

            nc.gpsimd.collective_compute(
                "AllGather", mybir.AluOpType.bypass, replica_groups=rg,
                ins=[ag1_in[:]], outs=[table1[:]],
            )

            # ---- aggregation (shared by both layers) ----
            def agg_layer(gbp, ohp, aps, table, col_off_mul, col_step, nfeat, dst_sb, layer, w_sb):
                off = 0
                for nb in range(NB):
                    psx = aps.tile([128, nfeat], F32, tag=f"psx{layer}")
                    nmm = sum(cnt[nb][s] // 128 for s in range(S))
                    mi = 0
                    for s in range(S):
                        cn = cnt[nb][s]
                        done = 0
                        while done < cn:
                            sub = min(1024, cn - done)
                            nch = sub // 128
                            gb = gbp.tile([128, 8, 64], F32, tag="gb")
                            nc.gpsimd.dma_gather(
                                gb[:, :nch, :],
                                table[:, col_off_mul * s : col_off_mul * s + 64],
                                eidx_sb[:, (off + done) // 16 : (off + done + sub) // 16],
                                num_idxs=sub,
                                num_idxs_reg=sub,
                                elem_size=64,
                                elem_step=col_step,
                            )
                            for ch in range(nch):
                                col = (off + done) // 128 + ch
                                oh = ohp.tile([128, 128], F32, tag="oh")
                                nc.vector.tensor_scalar(
                                    oh, iota_f,
                                    edst_sb[:, col : col + 1],
                                    w_sb[:, col : col + 1],
                                    mybir.AluOpType.is_equal,
                                    mybir.AluOpType.mult,
                                )
                                nc.tensor.matmul(
                                    psx, lhsT=oh, rhs=gb[:, ch, :nfeat],
                                    start=(mi == 0), stop=(mi == nmm - 1),
                                )
                                mi += 1
                            done += sub
                        off += cn
                    nc.scalar.activation(
                        dst_sb[:, nb, :], psx, mybir.ActivationFunctionType.Tanh
                    )

            with (
                tc.tile_pool(name="gbp", bufs=8) as gbp,
                tc.tile_pool(name="ohp", bufs=8) as ohp,
            ):
                with tc.tile_pool(name="aps1", bufs=2, space="PSUM") as aps1:
                    agg_layer(gbp, ohp, aps1, table1, H, 4 * H, H, x1_sb, 1, ew1_sb)

                # ---- layer-2 supports (V2 host-combined) ----
                with tc.tile_pool(name="s2ps", bufs=2, space="PSUM") as s2ps:
                    for nb in range(NB):
                        nsl = slice(128 * nb, 128 * (nb + 1))
                        ptx = s2ps.tile([H, 128], F32, tag="ptx")
                        nc.tensor.transpose(ptx, x1_sb[:, nb, :], ident)
                        nc.any.tensor_copy(x1t_sb[:, nsl], ptx)
                        ps2 = s2ps.tile([128, 4 * F], F32, tag="ps2")
                        nc.tensor.matmul(
                            ps2, lhsT=x1t_sb[:, nsl], rhs=v2_sb, start=True, stop=True
                        )
                        s2_sb = gbp.tile([128, 4 * F], F32, tag="s2sb")
                        nc.any.tensor_copy(s2_sb, ps2)
                        nc.sync.dma_start(ag2_in[nsl, : 4 * F], s2_sb)

                nc.gpsimd.collective_compute(
                    "AllGather", mybir.AluOpType.bypass, replica_groups=rg,
                    ins=[ag2_in[:]], outs=[table2[:]],
                )

                # ---- layer-2 aggregation ----
                with tc.tile_pool(name="aps2", bufs=2, space="PSUM") as aps2:
                    agg_layer(gbp, ohp, aps2, table2, F, T2COLS, F, x2_sb, 2, ew2_sb)

                # ---- classifier ----
                with tc.tile_pool(name="clfps", bufs=2, space="PSUM") as clfps:
                    for nb in range(NB):
                        nsl = slice(128 * nb, 128 * (nb + 1))
                        ptc = clfps.tile([F, 128], F32, tag="ptc")
                        nc.tensor.transpose(ptc, x2_sb[:, nb, :], ident)
                        x2t = gbp.tile([F, 128], F32, tag="x2t")
                        nc.any.tensor_copy(x2t, ptc)
                        pso = clfps.tile([C, 128], F32, tag="pso")
                        nc.tensor.matmul(pso, lhsT=wclf_sb, rhs=x2t, start=True, stop=True)
                        nc.vector.tensor_scalar(
                            out_sb[:, nsl], pso, bclf_sb[:, 0:1], None,
                            mybir.AluOpType.add,
                        )
                nc.sync.dma_start(out[:, :], out_sb)
    nc.finalize()
    return nc


def _prep_edges(edge_src, edge_dst, edge_w):
    """Bucket edges per (core, block, relation); pad to uniform chunk counts."""
    buckets = [[[None] * S for _ in range(NB)] for _ in range(NCORES)]
    for s in range(S):
        dst = edge_dst[s]
        core = dst // NPC
        blk = (dst % NPC) // 128
        dloc = dst % 128
        for c in range(NCORES):
            mc = core == c
            for b in range(NB):
                m = mc & (blk == b)
                buckets[c][b][s] = (
                    edge_src[s][m], dloc[m], edge_w[s][m]
                )
    cnt = [
        [
            ((max(len(buckets[c][b][s][0]) for c in range(NCORES)) + 127) // 128)
            * 128
            for s in range(S)
        ]
        for b in range(NB)
    ]
    tot = sum(cnt[b][s] for b in range(NB) for s in range(S))

    eidx_all, edst_all, ew_all = [], [], []
    for c in range(NCORES):
        src_st = np.zeros(tot, np.int16)
        dst_st = np.zeros(tot, np.uint8)
        w_st = np.zeros(tot, np.float32)
        off = 0
        for b in range(NB):
            for s in range(S):
                sr, dl, w = buckets[c][b][s]
                n = len(sr)
                src_st[off : off + n] = sr.astype(np.int16)
                dst_st[off : off + n] = dl.astype(np.uint8)
                w_st[off : off + n] = w
                off += cnt[b][s]
        eidx_all.append(np.ascontiguousarray(src_st.reshape(tot // 16, 16).T))
        edst_all.append(np.ascontiguousarray(dst_st.reshape(tot // 128, 128).T))
        ew_all.append(
            np.ascontiguousarray(w_st.reshape(tot // 128, 128).T.astype(np.float16))
        )
    return cnt, eidx_all, edst_all, ew_all


def _prep_inputs(features, edge_w, W1, Wc1, W2, Wc2, Wclf, bclf, edge_src, edge_dst):
    """Host prep: bucket edges, combine bases, transpose+bf16 features.
    Returns (cnt, in_maps)."""
    features = np.asarray(features, np.float32)
    edge_w = np.asarray(edge_w, np.float32)
    W1 = np.asarray(W1, np.float32)
    Wc1 = np.asarray(Wc1, np.float32)
    W2 = np.asarray(W2, np.float32)
    Wc2 = np.asarray(Wc2, np.float32)
    Wclf = np.asarray(Wclf, np.float32)
    bclf = np.asarray(bclf, np.float32)
    edge_src = np.asarray(edge_src, np.int32)
    edge_dst = np.asarray(edge_dst, np.int32)

    cnt, eidx_all, edst_all, ew_all = _prep_edges(edge_src, edge_dst, edge_w)

    # features: int12 fixed point (8-bit hi plane + packed 4-bit lo plane),
    # transposed, grouped per core so slices are contiguous
    amax = float(np.abs(features).max())
    delta = amax / 2047.5
    q = np.clip(np.round(features * (1.0 / delta)), -2048, 2047).astype(np.int16)
    qT_big = np.ascontiguousarray(
        q.T.reshape(N, NCORES, NPC).transpose(1, 0, 2)
    ).reshape(NCORES * N, NPC)
    fhi_big = (qT_big >> 4).astype(np.int8)
    nib = (qT_big & 15).astype(np.uint8)
    flo_big = nib[:, : NPC // 2] | (nib[:, NPC // 2 :] << 4)

    # host-side basis combination (small): V = Wc x W
    V1 = np.einsum("sb,bio->sio", Wc1, W1)              # [S, N, H]
    v1cat = np.concatenate([V1[s] for s in range(S)], axis=1).astype(NPFP16)
    V2 = np.einsum("sb,bio->sio", Wc2, W2)              # [S, H, F]
    v2cat = np.ascontiguousarray(
        np.concatenate([V2[s] for s in range(S)], axis=1).astype(np.float32)
    )

    tot = sum(cnt[b][s] for b in range(NB) for s in range(S))
    ncol = tot // 128
    eh = (tot // 16) // 2
    nch_half = ncol // 2
    in_maps = []
    for c in range(NCORES):
        m = dict(
            v2c=v2cat,
            wclf=Wclf,
            bc=bclf.reshape(C, 1),
            eidx0=np.ascontiguousarray(eidx_all[c][:, :eh]),
            eidx1=np.ascontiguousarray(eidx_all[c][:, eh:]),
            edst8=edst_all[c],
            ew16_0=np.ascontiguousarray(ew_all[c][:, :nch_half]),
            ew16_1=np.ascontiguousarray(ew_all[c][:, nch_half:]),
            v1s0=v1cat[c * NPC : c * NPC + NPC // 2],
            v1s1=v1cat[c * NPC + NPC // 2 : (c + 1) * NPC],
        )
        for j in range(NFH):
            m[f"fhi{j:02d}"] = fhi_big[c * N + j * RH : c * N + (j + 1) * RH]
        for j in range(NFL):
            m[f"flo{j:02d}"] = flo_big[c * N + j * RL : c * N + (j + 1) * RL]
        in_maps.append(m)
    return cnt, delta, in_maps


def kernel(features, edge_w, W1, Wc1, W2, Wc2, Wclf, bclf, edge_src, edge_dst):
    cnt, delta, in_maps = _prep_inputs(
        features, edge_w, W1, Wc1, W2, Wc2, Wclf, bclf, edge_src, edge_dst
    )
    nc = build_program(cnt, delta)
    res = run_bass_kernel_spmd(nc, in_maps, list(range(NCORES))).results
    return np.concatenate([res[c]["out"].T for c in range(NCORES)], axis=0)


# revision 41
# speedup vs baseline: 1.2145x; 1.2145x over previous
"""R-GCN (2-layer basis-decomposition GCN) on 8 Trainium2 NeuronCores.

The end-to-end time here is dominated by host->device input transfer over
the (axon-tunneled) PJRT link, so the design minimizes transferred bytes
and lets the device do all the math.

Strategy (1D node partition, per sharding hint):
- Nodes sharded 1024/core. The feature shard is sent TRANSPOSED as int12
  fixed point, split into an int8 hi plane + packed 4-bit lo plane
  (12MB/core vs 32MB fp32; the Gaussian hi plane additionally wire-
  compresses to ~0.85). The device reassembles q = 16*hi + nibble into
  fp16 (exact: |q| <= 2048) and runs the support matmul in fp16 with
  f32 PSUM accumulate. The dequant scale delta is folded into the
  layer-1 edge weights (aggregation is linear in w; tanh comes after),
  so no extra device pass is needed; layer 2 uses unscaled weights.
- The small basis combinations V1 = Wc1 x W1 ([8192, 256]) and
  V2 = Wc2 x W2 ([64, 128]) are computed on host. V1 is row-sharded
  (fp16, 0.5MB/core) and AllGathered on device; V2 is replicated (32KB).
- sup1 tables are AllGathered to Shared DRAM ([8192, 256] f32).
- Edges sharded by destination node, bucketed per (dst-block of 128,
  relation), padded to 128-edge chunks (pad: src=0, w=0). Edge index
  stream is sent untiled ([16, tot/16] int16) and replicated to the 128
  partitions on device (dma_gather wants indices wrapped in 16 partitions
  replicated across the 8 gpsimd cores); local dst rows as uint8;
  weights as fp16.
- Messages gathered with gpsimd.dma_gather (256B rows) landing as
  [128 edges (partitions), 64 feats] — directly the matmul moving operand.
- segment_sum = one-hot matmul: stationary [128e,128d] weighted one-hot
  built by one DVE tensor_scalar (iota is_equal dst) * w; PSUM
  accumulates per block.
- Layer 2 identical with a [8192,192] padded table; classifier on PE.

Accuracy: int12 features + fp16 V1 give rel err 5.5e-3 vs the f32
reference (gate 2e-2); the device pipeline reproduces the numpy
emulation of this quantization to 4 digits.
"""
import sys
import numpy as np

sys.path.insert(0, "/opt/trn_rl_repo")
import jax  # noqa: E402

# cache the XLA wrapper executable: run_bass_kernel_spmd re-jits per call
jax.config.update("jax_compilation_cache_dir", "/tmp/jaxcache")
jax.config.update("jax_persistent_cache_min_entry_size_bytes", 0)
jax.config.update("jax_persistent_cache_min_compile_time_secs", 0.0)

from concourse import bacc, bass, mybir, tile  # noqa: E402
from concourse.bass_utils import run_bass_kernel_spmd  # noqa: E402

FP16 = mybir.dt.float16
F32 = mybir.dt.float32
I16 = mybir.dt.int16
I32 = mybir.dt.int32
I8 = mybir.dt.int8
U8 = mybir.dt.uint8
NPFP16 = np.float16

N = 8192
S = 4
E = 262144
H = 64
F = 32
C = 2
NCORES = 8
NPC = N // NCORES      # 1024 nodes per core
NB = NPC // 128        # 8 dst blocks per core
KCH = N // 128         # 64 contraction chunks for layer 1
T2COLS = 192           # layer-2 table padded cols (768B rows)


def build_program(cnt, delta):
    """cnt: [NB][S] padded edge counts (identical across cores).
    delta: int12 feature quantization step (folded into layer-1 edge
    weights; layer 2 uses the unscaled weights)."""
    nc = bacc.Bacc(None)

    tot = sum(cnt[b][s] for b in range(NB) for s in range(S))
    ncol = tot // 128

    fhi = nc.dram_tensor("fhi", [N, NPC], I8, kind="ExternalInput")
    flo = nc.dram_tensor("flo", [N, NPC // 2], U8, kind="ExternalInput")
    v1s = nc.dram_tensor("v1s", [NPC, 4 * H], FP16, kind="ExternalInput")
    v2c = nc.dram_tensor("v2c", [H, 4 * F], F32, kind="ExternalInput")
    wclf = nc.dram_tensor("wclf", [F, C], F32, kind="ExternalInput")
    bc = nc.dram_tensor("bc", [C, 1], F32, kind="ExternalInput")
    eidx = nc.dram_tensor("eidx", [16, tot // 16], I16, kind="ExternalInput")
    edst8 = nc.dram_tensor("edst8", [128, ncol], U8, kind="ExternalInput")
    ew16 = nc.dram_tensor("ew16", [128, ncol], FP16, kind="ExternalInput")
    out = nc.dram_tensor("out", [C, NPC], F32, kind="ExternalOutput")

    agv1 = nc.dram_tensor("agv1", [NPC, 4 * H], FP16)
    tbv1 = nc.dram_tensor("tbv1", [N, 4 * H], FP16, addr_space="Shared")
    ag1_in = nc.dram_tensor("ag1_in", [NPC, 4 * H], F32)
    table1 = nc.dram_tensor("table1", [N, 4 * H], F32, addr_space="Shared")
    ag2_in = nc.dram_tensor("ag2_in", [NPC, T2COLS], F32)
    table2 = nc.dram_tensor("table2", [N, T2COLS], F32, addr_space="Shared")

    rg = [list(range(NCORES))]

    with tile.TileContext(nc) as tc:
        with tc.tile_pool(name="const", bufs=1) as cp:
            # ---- constants ----
            iota_i = cp.tile([128, 128], I32)
            nc.gpsimd.iota(iota_i, pattern=[[1, 128]], base=0, channel_multiplier=0)
            iota_f = cp.tile([128, 128], F32)
            nc.vector.tensor_copy(iota_f, iota_i)
            idn_i = cp.tile([128, 128], I32)
            nc.gpsimd.iota(idn_i, pattern=[[1, 128]], base=0, channel_multiplier=-1)
            ident = cp.tile([128, 128], F32)
            nc.vector.tensor_scalar(
                ident, idn_i, 0, None, mybir.AluOpType.is_equal
            )

            # edge streams: replicate idx block to all 8 gpsimd stripes
            eidx_sb = cp.tile([128, tot // 16], I16)
            for i in range(8):
                nc.sync.dma_start(eidx_sb[16 * i : 16 * (i + 1), :], eidx[:, :])
            edst8_sb = cp.tile([128, ncol], U8)
            nc.sync.dma_start(edst8_sb, edst8[:, :])
            edst_sb = cp.tile([128, ncol], F32)
            nc.vector.tensor_copy(edst_sb, edst8_sb)
            ew16_sb = cp.tile([128, ncol], FP16)
            nc.sync.dma_start(ew16_sb, ew16[:, :])
            # layer-1 weights absorb the int12 dequant scale; layer 2 unscaled
            ew1_sb = cp.tile([128, ncol], F32)
            nc.vector.tensor_scalar(
                ew1_sb, ew16_sb, float(delta), None, mybir.AluOpType.mult
            )
            ew2_sb = cp.tile([128, ncol], F32)
            nc.vector.tensor_copy(ew2_sb, ew16_sb)

            x1_sb = cp.tile([128, NB, H], F32)
            x1t_sb = cp.tile([H, NPC], F32)
            x2_sb = cp.tile([128, NB, F], F32)
            v2_sb = cp.tile([H, 4 * F], F32)
            nc.sync.dma_start(v2_sb, v2c[:, :])
            wclf_sb = cp.tile([F, C], F32)
            nc.sync.dma_start(wclf_sb, wclf[:, :])
            bclf_sb = cp.tile([C, 1], F32)
            nc.sync.dma_start(bclf_sb, bc[:, :])
            out_sb = cp.tile([C, NPC], F32)

            # ---- phase 0: AllGather V1 (host-combined, row-sharded) ----
            v1b = cp.tile([128, NB, 4 * H], FP16)
            for b in range(NB):
                nc.sync.dma_start(v1b[:, b, :], v1s[128 * b : 128 * (b + 1), :])
            for b in range(NB):
                nc.sync.dma_start(agv1[128 * b : 128 * (b + 1), :], v1b[:, b, :])
            nc.gpsimd.collective_compute(
                "AllGather", mybir.AluOpType.bypass, replica_groups=rg,
                ins=[agv1[:]], outs=[tbv1[:]],
            )
            v1 = cp.tile([128, KCH, 4 * H], FP16)
            for k in range(KCH):
                nc.sync.dma_start(v1[:, k, :], tbv1[128 * k : 128 * (k + 1), :])

            # ---- phase 1: support matmul sup1 = featT.T @ V1cat ----
            with (
                tc.tile_pool(name="ftp", bufs=3) as ftp,
                tc.tile_pool(name="spp", bufs=1, space="PSUM") as spp,
                tc.tile_pool(name="ssb", bufs=2) as ssb,
            ):
                ps = [
                    spp.tile([128, 4 * H], F32, tag=f"ps{b}", name=f"ps{b}")
                    for b in range(NB)
                ]
                hw = NPC // 2
                for k in range(KCH):
                    ksl = slice(128 * k, 128 * (k + 1))
                    hik = ftp.tile([128, NPC], I8, tag="hik")
                    nc.sync.dma_start(hik, fhi[ksl, :])
                    lok = ftp.tile([128, hw], U8, tag="lok")
                    nc.sync.dma_start(lok, flo[ksl, :])
                    # reassemble q = 16*hi + nibble into fp16 (q exact: |q|<=2048)
                    nib = ftp.tile([128, NPC], U8, tag="nib")
                    nc.vector.tensor_scalar(
                        nib[:, :hw], lok, 15, None, mybir.AluOpType.bitwise_and
                    )
                    nc.vector.tensor_scalar(
                        nib[:, hw:], lok, 4, None,
                        mybir.AluOpType.logical_shift_right,
                    )
                    ftk = ftp.tile([128, NPC], FP16, tag="ftk")
                    nc.vector.scalar_tensor_tensor(
                        ftk, hik, 16.0, nib,
                        mybir.AluOpType.mult, mybir.AluOpType.add,
                    )
                    for b in range(NB):
                        nc.tensor.matmul(
                            ps[b], lhsT=ftk[:, 128 * b : 128 * (b + 1)],
                            rhs=v1[:, k, :],
                            start=(k == 0), stop=(k == KCH - 1),
                        )
                for b in range(NB):
                    s_sb = ssb.tile([128, 4 * H], F32, tag="ssb")
                    nc.any.tensor_copy(s_sb, ps[b])
                    nc.sync.dma_start(ag1_in[128 * b : 128 * (b + 1), :], s_sb)

            nc.gpsimd.collective_compute(
                "AllGather", mybir.AluOpType.bypass, replica_groups=rg,
                ins=[ag1_in[:]], outs=[table1[:]],
            )

            # ---- aggregation (shared by both layers) ----
            def agg_layer(gbp, ohp, aps, table, col_off_mul, col_step, nfeat, dst_sb, layer, w_sb):
                off = 0
                for nb in range(NB):
                    psx = aps.tile([128, nfeat], F32, tag=f"psx{layer}")
                    nmm = sum(cnt[nb][s] // 128 for s in range(S))
                    mi = 0
                    for s in range(S):
                        cn = cnt[nb][s]
                        done = 0
                        while done < cn:
                            sub = min(1024, cn - done)
                            nch = sub // 128
                            gb = gbp.tile([128, 8, 64], F32, tag="gb")
                            nc.gpsimd.dma_gather(
                                gb[:, :nch, :],
                                table[:, col_off_mul * s : col_off_mul * s + 64],
                                eidx_sb[:, (off + done) // 16 : (off + done + sub) // 16],
                                num_idxs=sub,
                                num_idxs_reg=sub,
                                elem_size=64,
                                elem_step=col_step,
                            )
                            for ch in range(nch):
                                col = (off + done) // 128 + ch
                                oh = ohp.tile([128, 128], F32, tag="oh")
                                nc.vector.tensor_scalar(
                                    oh, iota_f,
                                    edst_sb[:, col : col + 1],
                                    w_sb[:, col : col + 1],
                                    mybir.AluOpType.is_equal,
                                    mybir.AluOpType.mult,
                                )
                                nc.tensor.matmul(
                                    psx, lhsT=oh, rhs=gb[:, ch, :nfeat],
                                    start=(mi == 0), stop=(mi == nmm - 1),
                                )
                                mi += 1
                            done += sub
                        off += cn
                    nc.scalar.activation(
                        dst_sb[:, nb, :], psx, mybir.ActivationFunctionType.Tanh
                    )

            with (
                tc.tile_pool(name="gbp", bufs=8) as gbp,
                tc.tile_pool(name="ohp", bufs=8) as ohp,
            ):
                with tc.tile_pool(name="aps1", bufs=2, space="PSUM") as aps1:
                    agg_layer(gbp, ohp, aps1, table1, H, 4 * H, H, x1_sb, 1, ew1_sb)

                # ---- layer-2 supports (V2 host-combined) ----
                with tc.tile_pool(name="s2ps", bufs=2, space="PSUM") as s2ps:
                    for nb in range(NB):
                        nsl = slice(128 * nb, 128 * (nb + 1))
                        ptx = s2ps.tile([H, 128], F32, tag="ptx")
                        nc.tensor.transpose(ptx, x1_sb[:, nb, :], ident)
                        nc.any.tensor_copy(x1t_sb[:, nsl], ptx)
                        ps2 = s2ps.tile([128, 4 * F], F32, tag="ps2")
                        nc.tensor.matmul(
                            ps2, lhsT=x1t_sb[:, nsl], rhs=v2_sb, start=True, stop=True
                        )
                        s2_sb = gbp.tile([128, 4 * F], F32, tag="s2sb")
                        nc.any.tensor_copy(s2_sb, ps2)
                        nc.sync.dma_start(ag2_in[nsl, : 4 * F], s2_sb)

                nc.gpsimd.collective_compute(
                    "AllGather", mybir.AluOpType.bypass, replica_groups=rg,
                    ins=[ag2_in[:]], outs=[table2[:]],
                )

                # ---- layer-2 aggregation ----
                with tc.tile_pool(name="aps2", bufs=2, space="PSUM") as aps2:
                    agg_layer(gbp, ohp, aps2, table2, F, T2COLS, F, x2_sb, 2, ew2_sb)

                # ---- classifier ----
                with tc.tile_pool(name="clfps", bufs=2, space="PSUM") as clfps:
                    for nb in range(NB):
                        nsl = slice(128 * nb, 128 * (nb + 1))
                        ptc = clfps.tile([F, 128], F32, tag="ptc")
                        nc.tensor.transpose(ptc, x2_sb[:, nb, :], ident)
                        x2t = gbp.tile([F, 128], F32, tag="x2t")
                        nc.any.tensor_copy(x2t, ptc)
                        pso = clfps.tile([C, 128], F32, tag="pso")
                        nc.tensor.matmul(pso, lhsT=wclf_sb, rhs=x2t, start=True, stop=True)
                        nc.vector.tensor_scalar(
                            out_sb[:, nsl], pso, bclf_sb[:, 0:1], None,
                            mybir.AluOpType.add,
                        )
                nc.sync.dma_start(out[:, :], out_sb)
    nc.finalize()
    return nc


def _prep_edges(edge_src, edge_dst, edge_w):
    """Bucket edges per (core, block, relation); pad to uniform chunk counts."""
    buckets = [[[None] * S for _ in range(NB)] for _ in range(NCORES)]
    for s in range(S):
        dst = edge_dst[s]
        core = dst // NPC
        blk = (dst % NPC) // 128
        dloc = dst % 128
        for c in range(NCORES):
            mc = core == c
            for b in range(NB):
                m = mc & (blk == b)
                buckets[c][b][s] = (
                    edge_src[s][m], dloc[m], edge_w[s][m]
                )
    cnt = [
        [
            ((max(len(buckets[c][b][s][0]) for c in range(NCORES)) + 127) // 128)
            * 128
            for s in range(S)
        ]
        for b in range(NB)
    ]
    tot = sum(cnt[b][s] for b in range(NB) for s in range(S))

    eidx_all, edst_all, ew_all = [], [], []
    for c in range(NCORES):
        src_st = np.zeros(tot, np.int16)
        dst_st = np.zeros(tot, np.uint8)
        w_st = np.zeros(tot, np.float32)
        off = 0
        for b in range(NB):
            for s in range(S):
                sr, dl, w = buckets[c][b][s]
                n = len(sr)
                src_st[off : off + n] = sr.astype(np.int16)
                dst_st[off : off + n] = dl.astype(np.uint8)
                w_st[off : off + n] = w
                off += cnt[b][s]
        eidx_all.append(np.ascontiguousarray(src_st.reshape(tot // 16, 16).T))
        edst_all.append(np.ascontiguousarray(dst_st.reshape(tot // 128, 128).T))
        ew_all.append(
            np.ascontiguousarray(w_st.reshape(tot // 128, 128).T.astype(np.float16))
        )
    return cnt, eidx_all, edst_all, ew_all


def _prep_inputs(features, edge_w, W1, Wc1, W2, Wc2, Wclf, bclf, edge_src, edge_dst):
    """Host prep: bucket edges, combine bases, transpose+bf16 features.
    Returns (cnt, in_maps)."""
    features = np.asarray(features, np.float32)
    edge_w = np.asarray(edge_w, np.float32)
    W1 = np.asarray(W1, np.float32)
    Wc1 = np.asarray(Wc1, np.float32)
    W2 = np.asarray(W2, np.float32)
    Wc2 = np.asarray(Wc2, np.float32)
    Wclf = np.asarray(Wclf, np.float32)
    bclf = np.asarray(bclf, np.float32)
    edge_src = np.asarray(edge_src, np.int32)
    edge_dst = np.asarray(edge_dst, np.int32)

    cnt, eidx_all, edst_all, ew_all = _prep_edges(edge_src, edge_dst, edge_w)

    # features: int12 fixed point (8-bit hi plane + packed 4-bit lo plane),
    # transposed, grouped per core so slices are contiguous
    amax = float(np.abs(features).max())
    delta = amax / 2047.5
    q = np.clip(np.round(features * (1.0 / delta)), -2048, 2047).astype(np.int16)
    qT_big = np.ascontiguousarray(
        q.T.reshape(N, NCORES, NPC).transpose(1, 0, 2)
    ).reshape(NCORES * N, NPC)
    fhi_big = (qT_big >> 4).astype(np.int8)
    nib = (qT_big & 15).astype(np.uint8)
    flo_big = nib[:, : NPC // 2] | (nib[:, NPC // 2 :] << 4)

    # host-side basis combination (small): V = Wc x W
    V1 = np.einsum("sb,bio->sio", Wc1, W1)              # [S, N, H]
    v1cat = np.concatenate([V1[s] for s in range(S)], axis=1).astype(NPFP16)
    V2 = np.einsum("sb,bio->sio", Wc2, W2)              # [S, H, F]
    v2cat = np.ascontiguousarray(
        np.concatenate([V2[s] for s in range(S)], axis=1).astype(np.float32)
    )

    in_maps = [
        dict(
            fhi=fhi_big[c * N : (c + 1) * N],
            flo=flo_big[c * N : (c + 1) * N],
            v1s=v1cat[c * NPC : (c + 1) * NPC],
            v2c=v2cat,
            wclf=Wclf,
            bc=bclf.reshape(C, 1),
            eidx=eidx_all[c],
            edst8=edst_all[c],
            ew16=ew_all[c],
        )
        for c in range(NCORES)
    ]
    return cnt, delta, in_maps


def kernel(features, edge_w, W1, Wc1, W2, Wc2, Wclf, bclf, edge_src, edge_dst):
    cnt, delta, in_maps = _prep_inputs(
        features, edge_w, W1, Wc1, W2, Wc2, Wclf, bclf, edge_src, edge_dst
    )
    nc = build_program(cnt, delta)
    res = run_bass_kernel_spmd(nc, in_maps, list(range(NCORES))).results
    return np.concatenate([res[c]["out"].T for c in range(NCORES)], axis=0)


# revision 42
# speedup vs baseline: 1.2203x; 1.0048x over previous
"""R-GCN (2-layer basis-decomposition GCN) on 8 Trainium2 NeuronCores.

The end-to-end time here is dominated by host->device input transfer over
the (axon-tunneled) PJRT link, so the design minimizes transferred bytes
and lets the device do all the math.

Strategy (1D node partition, per sharding hint):
- Nodes sharded 1024/core. The feature shard is sent TRANSPOSED as int12
  fixed point, split into an int8 hi plane + packed 4-bit lo plane
  (12MB/core vs 32MB fp32; the Gaussian hi plane additionally wire-
  compresses to ~0.85). The device reassembles q = 16*hi + nibble into
  fp16 (exact: |q| <= 2048) and runs the support matmul in fp16 with
  f32 PSUM accumulate. The dequant scale delta is folded into the
  layer-1 edge weights (aggregation is linear in w; tanh comes after),
  so no extra device pass is needed; layer 2 uses unscaled weights.
- The small basis combinations V1 = Wc1 x W1 ([8192, 256]) and
  V2 = Wc2 x W2 ([64, 128]) are computed on host. V1 is row-sharded
  (fp16, 0.5MB/core) and AllGathered on device; V2 is replicated (32KB).
- sup1 tables are AllGathered to Shared DRAM ([8192, 256] f32).
- Edges sharded by destination node, bucketed per (dst-block of 128,
  relation), padded to 128-edge chunks (pad: src=0, w=0). Edge index
  stream is sent untiled ([16, tot/16] int16) and replicated to the 128
  partitions on device (dma_gather wants indices wrapped in 16 partitions
  replicated across the 8 gpsimd cores); local dst rows as uint8;
  weights as fp16.
- Messages gathered with gpsimd.dma_gather (256B rows) landing as
  [128 edges (partitions), 64 feats] — directly the matmul moving operand.
- segment_sum = one-hot matmul: stationary [128e,128d] weighted one-hot
  built by one DVE tensor_scalar (iota is_equal dst) * w; PSUM
  accumulates per block.
- Layer 2 identical with a [8192,192] padded table; classifier on PE.

Accuracy: int12 features + fp16 V1 give rel err 5.5e-3 vs the f32
reference (gate 2e-2); the device pipeline reproduces the numpy
emulation of this quantization to 4 digits.
"""
import sys
import numpy as np

sys.path.insert(0, "/opt/trn_rl_repo")
import jax  # noqa: E402

# cache the XLA wrapper executable: run_bass_kernel_spmd re-jits per call
jax.config.update("jax_compilation_cache_dir", "/tmp/jaxcache")
jax.config.update("jax_persistent_cache_min_entry_size_bytes", 0)
jax.config.update("jax_persistent_cache_min_compile_time_secs", 0.0)

from concourse import bacc, bass, mybir, tile  # noqa: E402
from concourse.bass_utils import run_bass_kernel_spmd  # noqa: E402

FP16 = mybir.dt.float16
F32 = mybir.dt.float32
I16 = mybir.dt.int16
I32 = mybir.dt.int32
I8 = mybir.dt.int8
U8 = mybir.dt.uint8
NPFP16 = np.float16

N = 8192
S = 4
E = 262144
H = 64
F = 32
C = 2
NCORES = 8
NPC = N // NCORES      # 1024 nodes per core
NB = NPC // 128        # 8 dst blocks per core
KCH = N // 128         # 64 contraction chunks for layer 1
T2COLS = 192           # layer-2 table padded cols (768B rows)


def build_program(cnt, delta):
    """cnt: [NB][S] padded edge counts (identical across cores).
    delta: int12 feature quantization step (folded into layer-1 edge
    weights; layer 2 uses the unscaled weights)."""
    nc = bacc.Bacc(None)

    tot = sum(cnt[b][s] for b in range(NB) for s in range(S))
    ncol = tot // 128

    fhi = nc.dram_tensor("fhi", [N, NPC], I8, kind="ExternalInput")
    flo = nc.dram_tensor("flo", [N, NPC // 2], U8, kind="ExternalInput")
    v1s = nc.dram_tensor("v1s", [NPC, 4 * H], FP16, kind="ExternalInput")
    v2c = nc.dram_tensor("v2c", [H, 4 * F], F32, kind="ExternalInput")
    wclf = nc.dram_tensor("wclf", [F, C], F32, kind="ExternalInput")
    bc = nc.dram_tensor("bc", [C, 1], F32, kind="ExternalInput")
    eidx = nc.dram_tensor("eidx", [16, tot // 16], I16, kind="ExternalInput")
    edst8 = nc.dram_tensor("edst8", [128, ncol], U8, kind="ExternalInput")
    ew16 = nc.dram_tensor("ew16", [128, ncol], FP16, kind="ExternalInput")
    out = nc.dram_tensor("out", [C, NPC], F32, kind="ExternalOutput")

    agv1 = nc.dram_tensor("agv1", [NPC, 4 * H], FP16)
    tbv1 = nc.dram_tensor("tbv1", [N, 4 * H], FP16, addr_space="Shared")
    ag1_in = nc.dram_tensor("ag1_in", [NPC, 4 * H], F32)
    table1 = nc.dram_tensor("table1", [N, 4 * H], F32, addr_space="Shared")
    ag2_in = nc.dram_tensor("ag2_in", [NPC, T2COLS], F32)
    table2 = nc.dram_tensor("table2", [N, T2COLS], F32, addr_space="Shared")

    rg = [list(range(NCORES))]

    with tile.TileContext(nc) as tc:
        with tc.tile_pool(name="const", bufs=1) as cp:
            # ---- constants ----
            iota_i = cp.tile([128, 128], I32)
            nc.gpsimd.iota(iota_i, pattern=[[1, 128]], base=0, channel_multiplier=0)
            iota_f = cp.tile([128, 128], F32)
            nc.vector.tensor_copy(iota_f, iota_i)
            idn_i = cp.tile([128, 128], I32)
            nc.gpsimd.iota(idn_i, pattern=[[1, 128]], base=0, channel_multiplier=-1)
            ident = cp.tile([128, 128], F32)
            nc.vector.tensor_scalar(
                ident, idn_i, 0, None, mybir.AluOpType.is_equal
            )

            # edge streams: replicate idx block to all 8 gpsimd stripes
            eidx_sb = cp.tile([128, tot // 16], I16)
            for i in range(8):
                nc.sync.dma_start(eidx_sb[16 * i : 16 * (i + 1), :], eidx[:, :])
            edst8_sb = cp.tile([128, ncol], U8)
            nc.sync.dma_start(edst8_sb, edst8[:, :])
            edst_sb = cp.tile([128, ncol], F32)
            nc.vector.tensor_copy(edst_sb, edst8_sb)
            ew16_sb = cp.tile([128, ncol], FP16)
            nc.sync.dma_start(ew16_sb, ew16[:, :])
            # layer-1 weights absorb the int12 dequant scale; layer 2 unscaled
            ew1_sb = cp.tile([128, ncol], F32)
            nc.vector.tensor_scalar(
                ew1_sb, ew16_sb, float(delta), None, mybir.AluOpType.mult
            )
            ew2_sb = cp.tile([128, ncol], F32)
            nc.vector.tensor_copy(ew2_sb, ew16_sb)

            x1_sb = cp.tile([128, NB, H], F32)
            x1t_sb = cp.tile([H, NPC], F32)
            x2_sb = cp.tile([128, NB, F], F32)
            v2_sb = cp.tile([H, 4 * F], F32)
            nc.sync.dma_start(v2_sb, v2c[:, :])
            wclf_sb = cp.tile([F, C], F32)
            nc.sync.dma_start(wclf_sb, wclf[:, :])
            bclf_sb = cp.tile([C, 1], F32)
            nc.sync.dma_start(bclf_sb, bc[:, :])
            out_sb = cp.tile([C, NPC], F32)

            # ---- phase 0: AllGather V1 (host-combined, row-sharded) ----
            v1b = cp.tile([128, NB, 4 * H], FP16)
            for b in range(NB):
                nc.sync.dma_start(v1b[:, b, :], v1s[128 * b : 128 * (b + 1), :])
            for b in range(NB):
                nc.sync.dma_start(agv1[128 * b : 128 * (b + 1), :], v1b[:, b, :])
            nc.gpsimd.collective_compute(
                "AllGather", mybir.AluOpType.bypass, replica_groups=rg,
                ins=[agv1[:]], outs=[tbv1[:]],
            )
            v1 = cp.tile([128, KCH, 4 * H], FP16)
            for k in range(KCH):
                nc.sync.dma_start(v1[:, k, :], tbv1[128 * k : 128 * (k + 1), :])

            # ---- phase 1: support matmul sup1 = featT.T @ V1cat ----
            with (
                tc.tile_pool(name="ftp", bufs=3) as ftp,
                tc.tile_pool(name="spp", bufs=1, space="PSUM") as spp,
                tc.tile_pool(name="ssb", bufs=2) as ssb,
            ):
                ps = [
                    spp.tile([128, 4 * H], F32, tag=f"ps{b}", name=f"ps{b}")
                    for b in range(NB)
                ]
                hw = NPC // 2
                for k in range(KCH):
                    ksl = slice(128 * k, 128 * (k + 1))
                    hik = ftp.tile([128, NPC], I8, tag="hik")
                    nc.sync.dma_start(hik, fhi[ksl, :])
                    lok = ftp.tile([128, hw], U8, tag="lok")
                    nc.sync.dma_start(lok, flo[ksl, :])
                    # reassemble q = 16*hi + nibble into fp16 (q exact: |q|<=2048)
                    nib = ftp.tile([128, NPC], U8, tag="nib")
                    nc.vector.tensor_scalar(
                        nib[:, :hw], lok, 15, None, mybir.AluOpType.bitwise_and
                    )
                    nc.vector.tensor_scalar(
                        nib[:, hw:], lok, 4, None,
                        mybir.AluOpType.logical_shift_right,
                    )
                    ftk = ftp.tile([128, NPC], FP16, tag="ftk")
                    nc.vector.scalar_tensor_tensor(
                        ftk, hik, 16.0, nib,
                        mybir.AluOpType.mult, mybir.AluOpType.add,
                    )
                    for b in range(NB):
                        nc.tensor.matmul(
                            ps[b], lhsT=ftk[:, 128 * b : 128 * (b + 1)],
                            rhs=v1[:, k, :],
                            start=(k == 0), stop=(k == KCH - 1),
                        )
                for b in range(NB):
                    s_sb = ssb.tile([128, 4 * H], F32, tag="ssb")
                    nc.any.tensor_copy(s_sb, ps[b])
                    nc.sync.dma_start(ag1_in[128 * b : 128 * (b + 1), :], s_sb)

            nc.gpsimd.collective_compute(
                "AllGather", mybir.AluOpType.bypass, replica_groups=rg,
                ins=[ag1_in[:]], outs=[table1[:]],
            )

            # ---- aggregation (shared by both layers) ----
            def agg_layer(gbp, ohp, aps, table, col_off_mul, col_step, nfeat, dst_sb, layer, w_sb):
                off = 0
                for nb in range(NB):
                    psx = aps.tile([128, nfeat], F32, tag=f"psx{layer}")
                    nmm = sum(cnt[nb][s] // 128 for s in range(S))
                    mi = 0
                    for s in range(S):
                        cn = cnt[nb][s]
                        done = 0
                        while done < cn:
                            sub = min(1024, cn - done)
                            nch = sub // 128
                            gb = gbp.tile([128, 8, 64], F32, tag="gb")
                            nc.gpsimd.dma_gather(
                                gb[:, :nch, :],
                                table[:, col_off_mul * s : col_off_mul * s + 64],
                                eidx_sb[:, (off + done) // 16 : (off + done + sub) // 16],
                                num_idxs=sub,
                                num_idxs_reg=sub,
                                elem_size=64,
                                elem_step=col_step,
                            )
                            for ch in range(nch):
                                col = (off + done) // 128 + ch
                                oh = ohp.tile([128, 128], F32, tag="oh")
                                nc.vector.tensor_scalar(
                                    oh, iota_f,
                                    edst_sb[:, col : col + 1],
                                    w_sb[:, col : col + 1],
                                    mybir.AluOpType.is_equal,
                                    mybir.AluOpType.mult,
                                )
                                nc.tensor.matmul(
                                    psx, lhsT=oh, rhs=gb[:, ch, :nfeat],
                                    start=(mi == 0), stop=(mi == nmm - 1),
                                )
                                mi += 1
                            done += sub
                        off += cn
                    nc.scalar.activation(
                        dst_sb[:, nb, :], psx, mybir.ActivationFunctionType.Tanh
                    )

            with (
                tc.tile_pool(name="gbp", bufs=8) as gbp,
                tc.tile_pool(name="ohp", bufs=8) as ohp,
            ):
                with tc.tile_pool(name="aps1", bufs=2, space="PSUM") as aps1:
                    agg_layer(gbp, ohp, aps1, table1, H, 4 * H, H, x1_sb, 1, ew1_sb)

                # ---- layer-2 supports (V2 host-combined) ----
                with tc.tile_pool(name="s2ps", bufs=2, space="PSUM") as s2ps:
                    for nb in range(NB):
                        nsl = slice(128 * nb, 128 * (nb + 1))
                        ptx = s2ps.tile([H, 128], F32, tag="ptx")
                        nc.tensor.transpose(ptx, x1_sb[:, nb, :], ident)
                        nc.any.tensor_copy(x1t_sb[:, nsl], ptx)
                        ps2 = s2ps.tile([128, 4 * F], F32, tag="ps2")
                        nc.tensor.matmul(
                            ps2, lhsT=x1t_sb[:, nsl], rhs=v2_sb, start=True, stop=True
                        )
                        s2_sb = gbp.tile([128, 4 * F], F32, tag="s2sb")
                        nc.any.tensor_copy(s2_sb, ps2)
                        nc.sync.dma_start(ag2_in[nsl, : 4 * F], s2_sb)

                nc.gpsimd.collective_compute(
                    "AllGather", mybir.AluOpType.bypass, replica_groups=rg,
                    ins=[ag2_in[:]], outs=[table2[:]],
                )

                # ---- layer-2 aggregation ----
                with tc.tile_pool(name="aps2", bufs=2, space="PSUM") as aps2:
                    agg_layer(gbp, ohp, aps2, table2, F, T2COLS, F, x2_sb, 2, ew2_sb)

                # ---- classifier ----
                with tc.tile_pool(name="clfps", bufs=2, space="PSUM") as clfps:
                    for nb in range(NB):
                        nsl = slice(128 * nb, 128 * (nb + 1))
                        ptc = clfps.tile([F, 128], F32, tag="ptc")
                        nc.tensor.transpose(ptc, x2_sb[:, nb, :], ident)
                        x2t = gbp.tile([F, 128], F32, tag="x2t")
                        nc.any.tensor_copy(x2t, ptc)
                        pso = clfps.tile([C, 128], F32, tag="pso")
                        nc.tensor.matmul(pso, lhsT=wclf_sb, rhs=x2t, start=True, stop=True)
                        nc.vector.tensor_scalar(
                            out_sb[:, nsl], pso, bclf_sb[:, 0:1], None,
                            mybir.AluOpType.add,
                        )
                nc.sync.dma_start(out[:, :], out_sb)
    nc.finalize()
    # memoize the BIR serialization: the program is immutable after
    # finalize, but _bass_exec lowering re-serializes it (8.3MB JSON,
    # ~65ms) on every run_bass_kernel_spmd call
    blob = nc.to_json_bytes()
    nc.to_json_bytes = lambda: blob
    return nc


def _prep_edges(edge_src, edge_dst, edge_w):
    """Bucket edges per (core, block, relation); pad to uniform chunk counts."""
    buckets = [[[None] * S for _ in range(NB)] for _ in range(NCORES)]
    for s in range(S):
        dst = edge_dst[s]
        core = dst // NPC
        blk = (dst % NPC) // 128
        dloc = dst % 128
        for c in range(NCORES):
            mc = core == c
            for b in range(NB):
                m = mc & (blk == b)
                buckets[c][b][s] = (
                    edge_src[s][m], dloc[m], edge_w[s][m]
                )
    cnt = [
        [
            ((max(len(buckets[c][b][s][0]) for c in range(NCORES)) + 127) // 128)
            * 128
            for s in range(S)
        ]
        for b in range(NB)
    ]
    tot = sum(cnt[b][s] for b in range(NB) for s in range(S))

    eidx_all, edst_all, ew_all = [], [], []
    for c in range(NCORES):
        src_st = np.zeros(tot, np.int16)
        dst_st = np.zeros(tot, np.uint8)
        w_st = np.zeros(tot, np.float32)
        off = 0
        for b in range(NB):
            for s in range(S):
                sr, dl, w = buckets[c][b][s]
                n = len(sr)
                src_st[off : off + n] = sr.astype(np.int16)
                dst_st[off : off + n] = dl.astype(np.uint8)
                w_st[off : off + n] = w
                off += cnt[b][s]
        eidx_all.append(np.ascontiguousarray(src_st.reshape(tot // 16, 16).T))
        edst_all.append(np.ascontiguousarray(dst_st.reshape(tot // 128, 128).T))
        ew_all.append(
            np.ascontiguousarray(w_st.reshape(tot // 128, 128).T.astype(np.float16))
        )
    return cnt, eidx_all, edst_all, ew_all


def _prep_inputs(features, edge_w, W1, Wc1, W2, Wc2, Wclf, bclf, edge_src, edge_dst):
    """Host prep: bucket edges, combine bases, transpose+bf16 features.
    Returns (cnt, in_maps)."""
    features = np.asarray(features, np.float32)
    edge_w = np.asarray(edge_w, np.float32)
    W1 = np.asarray(W1, np.float32)
    Wc1 = np.asarray(Wc1, np.float32)
    W2 = np.asarray(W2, np.float32)
    Wc2 = np.asarray(Wc2, np.float32)
    Wclf = np.asarray(Wclf, np.float32)
    bclf = np.asarray(bclf, np.float32)
    edge_src = np.asarray(edge_src, np.int32)
    edge_dst = np.asarray(edge_dst, np.int32)

    cnt, eidx_all, edst_all, ew_all = _prep_edges(edge_src, edge_dst, edge_w)

    # features: int12 fixed point (8-bit hi plane + packed 4-bit lo plane),
    # transposed, grouped per core so slices are contiguous
    amax = float(np.abs(features).max())
    delta = amax / 2047.5
    q = np.clip(np.round(features * (1.0 / delta)), -2048, 2047).astype(np.int16)
    qT_big = np.ascontiguousarray(
        q.T.reshape(N, NCORES, NPC).transpose(1, 0, 2)
    ).reshape(NCORES * N, NPC)
    fhi_big = (qT_big >> 4).astype(np.int8)
    nib = (qT_big & 15).astype(np.uint8)
    flo_big = nib[:, : NPC // 2] | (nib[:, NPC // 2 :] << 4)

    # host-side basis combination (small): V = Wc x W
    V1 = np.einsum("sb,bio->sio", Wc1, W1)              # [S, N, H]
    v1cat = np.concatenate([V1[s] for s in range(S)], axis=1).astype(NPFP16)
    V2 = np.einsum("sb,bio->sio", Wc2, W2)              # [S, H, F]
    v2cat = np.ascontiguousarray(
        np.concatenate([V2[s] for s in range(S)], axis=1).astype(np.float32)
    )

    in_maps = [
        dict(
            fhi=fhi_big[c * N : (c + 1) * N],
            flo=flo_big[c * N : (c + 1) * N],
            v1s=v1cat[c * NPC : (c + 1) * NPC],
            v2c=v2cat,
            wclf=Wclf,
            bc=bclf.reshape(C, 1),
            eidx=eidx_all[c],
            edst8=edst_all[c],
            ew16=ew_all[c],
        )
        for c in range(NCORES)
    ]
    return cnt, delta, in_maps


def kernel(features, edge_w, W1, Wc1, W2, Wc2, Wclf, bclf, edge_src, edge_dst):
    cnt, delta, in_maps = _prep_inputs(
        features, edge_w, W1, Wc1, W2, Wc2, Wclf, bclf, edge_src, edge_dst
    )
    nc = build_program(cnt, delta)
    res = run_bass_kernel_spmd(nc, in_maps, list(range(NCORES))).results
    return np.concatenate([res[c]["out"].T for c in range(NCORES)], axis=0)
